# revision 24
# baseline (speedup 1.0000x reference)
"""DPOT2D layer (AFNO-style) Trainium2 kernel.

out = x + irfft2_pad(blockMLP(trunc64(rfft2(x))))   (ortho norm)

Sharding: tensor-parallel over the 8 block-diagonal channel groups — core n
gets channels [n*64, (n+1)*64) and its block's MLP weights. Blocks never mix,
so there is zero cross-core communication.

Per core, every FFT stage is a DFT matmul on the TensorEngine (bf16 operands,
fp32 PSUM accumulation), with PE-transpose corner turns between stages:

  A:  V[w,(c,k1s)]   = x-slice^T @ F_h     (activation-stationary: x[h, c, w]
                                            slices are lhsT, contract h; no
                                            corner turn needed before B)
  B:  Y[k2s,(c,k1)]  = DFT_w on complex V  (re/im column accumulation)
  t2: Yt[c,(k1,k2s)] = corner turn of Y
  L1: o1 = gelu(W1 Y + b1)                 (K=64, re/im col accumulation)
  L2: O2[(o2r|o2i),(k1,k2)] = W2 o1 + b2   (K=128)
  t3: R[k2,(k1,o2s)] = corner turn of O2
  iW: G[w,(j,k1,c)]  = hermitian irfft_w matmuls (re/im col accumulation)
  t4: Ght[k1s,(w,c)] = corner turn of G (re/im interleaved -> k1-stack)
  iH: x'[h,(w,c)]    = E_h^T @ Ght, + residual x (bf16 from SBUF), DMA out

The residual is the bf16 x already resident in SBUF (loaded once per batch as
two whole-plane [128, 256, 64] tiles), so HBM traffic is 2x8.4MB in +
2x8.4MB out bf16 per core. Output DRAM is bf16; the host upcasts to f32.
PSUM->SBUF drains use 2-bank fp32 tiles (FD=1024) for matmul stages and
bf16 1-bank tiles (FD=512, 2x copy mode) for the corner turns, statically
balanced between DVE and ACT.
"""

import numpy as np
import ml_dtypes

import concourse.bass as bass
import concourse.mybir as mybir
from concourse import bacc
from concourse import masks
from concourse.tile import TileContext
from concourse.bass_utils import run_bass_kernel_spmd

B = 2
H = 256
W = 256
C = 512
NB = 8
BS = 64          # channels per block (= per core)
KEEP = 64        # kept modes per spatial dim
HID = 128
P = 128

BF16 = mybir.dt.bfloat16
F32 = mybir.dt.float32
AF = mybir.ActivationFunctionType

_CACHED_NC = None


def _host_consts():
    """DFT matrices shared by all cores (fp32 -> bf16)."""
    h = np.arange(H, dtype=np.float64)[:, None]
    k = np.arange(KEEP, dtype=np.float64)[None, :]
    th = 2.0 * np.pi * h * k / H
    F = np.concatenate([np.cos(th), -np.sin(th)], axis=1) / 16.0      # (256,128)
    Fwre, Fwim = F[:, :KEEP], F[:, KEEP:]
    lb_re = np.concatenate([Fwre, Fwim], axis=1)                      # (256,128)
    lb_im = np.concatenate([-Fwim, Fwre], axis=1)
    alpha = np.where(np.arange(KEEP) == 0, 1.0, 2.0)
    k2 = np.arange(KEEP, dtype=np.float64)[:, None]
    wv = np.arange(W, dtype=np.float64)[None, :]
    tw = 2.0 * np.pi * k2 * wv / W
    Ca = alpha[:, None] * np.cos(tw) / 16.0                           # (64,256)
    Sa = alpha[:, None] * np.sin(tw) / 16.0
    k1 = np.arange(KEEP, dtype=np.float64)[:, None]
    hv = np.arange(H, dtype=np.float64)[None, :]
    tih = 2.0 * np.pi * k1 * hv / H
    Ehc = np.cos(tih) / 16.0                                          # (64,256)
    Ehs = np.sin(tih) / 16.0
    lih_full = np.concatenate([Ehc, -Ehs], axis=0)                    # (128,256)

    bf = ml_dtypes.bfloat16
    ffwd = np.stack([F[0:128], F[128:256]]).astype(bf)                # (2,128,128)
    lbw = np.stack([
        np.stack([lb_re[0:128], lb_im[0:128]]),
        np.stack([lb_re[128:256], lb_im[128:256]]),
    ]).astype(bf)                                                     # (2,2,128,128)
    liw = np.stack([
        np.stack([np.stack([Ca[:, 0:128], -Sa[:, 0:128]]),
                  np.stack([Sa[:, 0:128], Ca[:, 0:128]])]),
        np.stack([np.stack([Ca[:, 128:256], -Sa[:, 128:256]]),
                  np.stack([Sa[:, 128:256], Ca[:, 128:256]])]),
    ]).astype(bf)                                                     # (2,2,2,64,128)
    lih = np.stack([lih_full[:, 0:128], lih_full[:, 128:256]]).astype(bf)  # (2,128,128)
    return ffwd, lbw, liw, lih


def _build_nc(loop_iters=0, probe=None):
    """loop_iters>0 wraps the whole per-batch pipeline in an on-device
    For_i repeat loop — used only by the timing harness to amortize the
    axon dispatch overhead out of the measurement.
    probe: None | 'dma' (DMAs only) | 'compute' (no input DMAs)."""
    nc = bacc.Bacc()

    xbf = nc.declare_dram_parameter("xbf", [B, H, BS, W], BF16, isOutput=False)
    ffwd_d = nc.declare_dram_parameter("ffwd", [2, P, P], BF16, isOutput=False)
    lbw_d = nc.declare_dram_parameter("lbw", [2, 2, P, P], BF16, isOutput=False)
    m1_d = nc.declare_dram_parameter("m1", [2, 2, BS, HID], BF16, isOutput=False)
    m2_d = nc.declare_dram_parameter("m2", [2, HID, P], BF16, isOutput=False)
    b1s_d = nc.declare_dram_parameter("b1s", [2, HID, 1], F32, isOutput=False)
    b2s_d = nc.declare_dram_parameter("b2s", [P, 1], F32, isOutput=False)
    liw_d = nc.declare_dram_parameter("liw", [2, 2, 2, KEEP, P], BF16, isOutput=False)
    lih_d = nc.declare_dram_parameter("lih", [2, P, P], BF16, isOutput=False)
    out = nc.declare_dram_parameter("out", [B, H, W, BS], BF16, isOutput=True)

    with TileContext(nc) as tc:
        consts = tc.alloc_tile_pool(name="consts", bufs=1)
        ident = consts.tile([P, P], BF16, name="ident")
        masks.make_identity(nc, ident[:])

        def const2d(name, dram_ap, shape, dtype=BF16):
            t = consts.tile(shape, dtype, name=name)
            nc.sync.dma_start(out=t[:], in_=dram_ap)
            return t

        FW = [const2d(f"fw{hh}", ffwd_d[hh], [P, P]) for hh in range(2)]
        LBW = [[const2d(f"lbw{wh}{s}", lbw_d[wh, s], [P, P]) for s in range(2)]
               for wh in range(2)]
        M1 = [[const2d(f"m1_{j}{s}", m1_d[j, s], [BS, HID]) for s in range(2)]
              for j in range(2)]
        M2 = [const2d(f"m2_{s}", m2_d[s], [HID, P]) for s in range(2)]
        LIW = [[[const2d(f"liw{wh}{j}{s}", liw_d[wh, j, s], [KEEP, P])
                 for s in range(2)] for j in range(2)] for wh in range(2)]
        LIH = [const2d(f"lih{hc}", lih_d[hc], [P, P]) for hc in range(2)]
        b1s_t = [const2d(f"b1s{j}", b1s_d[j], [HID, 1], F32) for j in range(2)]
        b2s_t = const2d("b2s", b2s_d[:], [P, 1], F32)

        # PSUM->SBUF drain with static DVE/ACT balance. kind:
        #   'copy'  — plain copy (tensor_copy / activation Copy)
        #   engines chosen by per-stage pattern strings of 'D'/'A'.
        def mk_cp():
            state = {}

            def cp(dst, src, pat, key):
                i = state.get(key, 0)
                state[key] = i + 1
                if pat[i % len(pat)] == "D":
                    nc.vector.tensor_copy(out=dst, in_=src)
                else:
                    nc.scalar.activation(out=dst, in_=src, func=AF.Copy)
            return cp

        cp = mk_cp()

        sb = tc.alloc_tile_pool(name="sb", bufs=1)
        xtp = tc.alloc_tile_pool(name="xtp", bufs=1)
        outp = tc.alloc_tile_pool(name="outp", bufs=4)
        pmm = tc.alloc_tile_pool(name="pmm", bufs=2, space="PSUM")
        ptp = tc.alloc_tile_pool(name="ptp", bufs=4, space="PSUM")

        import contextlib
        loop_ctx = tc.For_i(0, loop_iters, 1) if loop_iters else contextlib.nullcontext()
        with loop_ctx:
            if probe == "dma":
                _emit_dma_probe(nc, tc, locals())
            else:
                _emit_body(nc, tc, locals(), skip_dma=(probe == "compute"))
        ptp.release()
        pmm.release()
        outp.release()
        xtp.release()
        sb.release()
        consts.release()
    nc.compile()
    return nc


def _emit_dma_probe(nc, tc, env):
    """Same DMA traffic as the real kernel (x-in bf16 whole planes, out bf16),
    no compute: out tiles are fed from the x tiles."""
    xbf = env["xbf"]; out = env["out"]
    xtp = env["xtp"]; outp = env["outp"]
    for b in range(B):
        xt = []
        for hh in range(2):
            t = xtp.tile([P, BS, W], BF16, tag=f"xt{hh}", name=f"pxt{hh}_{b}")
            nc.sync.dma_start(out=t[:], in_=xbf[b, hh * P:(hh + 1) * P, :, :])
            xt.append(t)
        for hc in range(2):
            for q8 in range(8):
                ot = outp.tile([P, 32, BS], BF16, tag="ot",
                               name=f"pot_{b}_{hc}_{q8}")
                nc.vector.tensor_copy(
                    out=ot[:],
                    in_=xt[hc][:, :, q8 * 32:(q8 + 1) * 32].rearrange(
                        "p c w -> p w c"))
                nc.sync.dma_start(
                    out=out[b, hc * P:(hc + 1) * P, q8 * 32:(q8 + 1) * 32, :],
                    in_=ot[:])


def _emit_body(nc, tc, env, skip_dma=False):
    xbf = env["xbf"]; out = env["out"]
    FW = env["FW"]; LBW = env["LBW"]; M1 = env["M1"]; M2 = env["M2"]
    LIW = env["LIW"]; LIH = env["LIH"]; b1s_t = env["b1s_t"]; b2s_t = env["b2s_t"]
    ident = env["ident"]; cp = env["cp"]
    sb = env["sb"]; xtp = env["xtp"]; outp = env["outp"]
    pmm = env["pmm"]; ptp = env["ptp"]

    EV = "A"             # matmul-stage drains: ACT (DVE is loaded by iH adds)
    SK = "DADDA"         # turn drains: 3 DVE / 2 ACT (bf16 src -> DVE 2x mode)

    for b in range(B):
        # ---------------- load x: two whole (c,w) planes ----------------
        # Chunked by c-range and hh-interleaved so stage A (which consumes
        # c in order, both hh per accumulation) starts after the first ~1/4
        # of the load instead of waiting for both full planes — the load is
        # WAR-serialized behind the previous batch's residual reads, so its
        # head latency is otherwise fully exposed.
        xt = [xtp.tile([P, BS, W], BF16, tag=f"xt{hh}", name=f"xt{hh}_{b}")
              for hh in range(2)]
        for q in range(4):
            for hh in range(2):
                if not skip_dma:
                    nc.sync.dma_start(
                        out=xt[hh][:, q * 16:(q + 1) * 16, :],
                        in_=xbf[b, hh * P:(hh + 1) * P, q * 16:(q + 1) * 16, :])
                elif q == 0:
                    nc.sync.dma_start(out=xt[hh][0:1, 0:1, :],
                                      in_=xbf[b, 0:1, 0:1, :])

        # ------- stage A (act-stationary): V[wh] (128=w, (c 64, k1s 128)) --
        # lhsT = x[h, c, w-half] slice (contract h), rhs = F_h chunk.
        V = [sb.tile([P, BS, P], BF16, tag=f"tagCD{wh}", name=f"V{wh}_{b}")
             for wh in range(2)]
        for wh in range(2):
            for cq in range(8):      # 8 c per psum tile
                ps = pmm.tile([P, 8, P], F32, tag="mm", name=f"psA_{b}_{wh}_{cq}")
                for i in range(8):
                    c = cq * 8 + i
                    for hh in range(2):
                        nc.tensor.matmul(
                            ps[:, i, :],
                            xt[hh][:, c, wh * P:(wh + 1) * P], FW[hh],
                            start=(hh == 0), stop=(hh == 1))
                cp(V[wh][:, cq * 8:(cq + 1) * 8, :], ps[:], EV, "A")

        # ---------------- stage B: Y (128=k2s, (c 64, k1 64)) --------------
        Y = sb.tile([P, BS, KEEP], BF16, tag="tagE", name=f"Y_{b}")
        for np_ in range(4):         # 16 c per psum tile
            ps = pmm.tile([P, 16, KEEP], F32, tag="mm", name=f"psB_{b}_{np_}")
            for k in range(2):
                nn = np_ * 2 + k     # 8-c chunk
                first = True
                for wh in range(2):
                    for s in range(2):
                        rhs = V[wh][:, nn * 8:(nn + 1) * 8,
                                    s * KEEP:(s + 1) * KEEP]
                        nc.tensor.matmul(ps[:, k * 8:(k + 1) * 8, :],
                                         LBW[wh][s], rhs,
                                         start=first, stop=(wh == 1 and s == 1))
                        first = False
            cp(Y[:, np_ * 16:(np_ + 1) * 16, :], ps[:], EV, "B")

        # ---------------- turn2: Yt (64=c, (k1 64, k2s 128)) ---------------
        Yt = sb.tile([BS, KEEP, P], BF16, tag="tagF", name=f"Yt_{b}")
        for kq in range(16):         # groups of 4 k1
            pt = ptp.tile([P, 4, P], BF16, tag="tp", name=f"t2_{b}_{kq}")
            for i in range(4):
                nc.tensor.transpose(pt[0:BS, i, :], Y[:, :, kq * 4 + i],
                                    ident[:])
            cp(Yt[:, kq * 4:(kq + 1) * 4, :], pt[0:BS, :, :], SK, "t2")

        # ---------------- MLP L1 (K=64) + gelu -----------------------------
        o1 = [sb.tile([HID, KEEP, KEEP], BF16, tag=f"o1_{j}", name=f"o1_{j}_{b}")
              for j in range(2)]
        for j in range(2):
            for np_ in range(4):     # 16 k1 per psum tile
                ps = pmm.tile([HID, 16, KEEP], F32, tag="mm",
                              name=f"ps1_{b}_{j}_{np_}")
                for k in range(2):
                    nn = np_ * 2 + k
                    nc.tensor.matmul(
                        ps[:, k * 8:(k + 1) * 8, :], M1[j][0],
                        Yt[:, nn * 8:(nn + 1) * 8, 0:KEEP],
                        start=True, stop=False)
                    nc.tensor.matmul(
                        ps[:, k * 8:(k + 1) * 8, :], M1[j][1],
                        Yt[:, nn * 8:(nn + 1) * 8, KEEP:P],
                        start=False, stop=True)
                nc.scalar.activation(
                    out=o1[j][:, np_ * 16:(np_ + 1) * 16, :],
                    in_=ps[:], func=AF.Gelu, bias=b1s_t[j][:])

        # ---------------- MLP L2 (K=128) + bias ----------------------------
        O2 = sb.tile([P, KEEP, KEEP], BF16, tag="tagF2", name=f"O2_{b}")
        for np_ in range(4):
            ps = pmm.tile([P, 16, KEEP], F32, tag="mm", name=f"ps2_{b}_{np_}")
            for k in range(2):
                nn = np_ * 2 + k
                nc.tensor.matmul(ps[:, k * 8:(k + 1) * 8, :], M2[0],
                                 o1[0][:, nn * 8:(nn + 1) * 8, :],
                                 start=True, stop=False)
                nc.tensor.matmul(ps[:, k * 8:(k + 1) * 8, :], M2[1],
                                 o1[1][:, nn * 8:(nn + 1) * 8, :],
                                 start=False, stop=True)
            nc.scalar.activation(
                out=O2[:, np_ * 16:(np_ + 1) * 16, :], in_=ps[:],
                func=AF.Identity, bias=b2s_t[:])

        # ---------------- turn3: R (64=k2, (k1 64, o2s 128)) ---------------
        R = sb.tile([KEEP, KEEP, P], BF16, tag="tagE", name=f"R_{b}")
        for kq in range(16):
            pt = ptp.tile([P, 4, P], BF16, tag="tp", name=f"t3_{b}_{kq}")
            for i in range(4):
                nc.tensor.transpose(pt[0:KEEP, i, :], O2[:, kq * 4 + i, :],
                                    ident[:])
            cp(R[:, kq * 4:(kq + 1) * 4, :], pt[0:KEEP, :, :], SK, "t3")

        # ---------------- invW: G[wh] (128=w, (j 2, k1 64, c 64)) ----------
        G = [sb.tile([P, 2, KEEP, BS], BF16, tag=f"tagAB{wh}", name=f"G{wh}_{b}")
             for wh in range(2)]
        for wh in range(2):
            for j in range(2):       # 0: Gre, 1: Gim
                for np_ in range(4):
                    ps = pmm.tile([P, 16, BS], F32, tag="mm",
                                  name=f"psW_{b}_{wh}_{j}_{np_}")
                    for k in range(2):
                        nn = np_ * 2 + k
                        nc.tensor.matmul(
                            ps[:, k * 8:(k + 1) * 8, :], LIW[wh][j][0],
                            R[:, nn * 8:(nn + 1) * 8, 0:KEEP],
                            start=True, stop=False)
                        nc.tensor.matmul(
                            ps[:, k * 8:(k + 1) * 8, :], LIW[wh][j][1],
                            R[:, nn * 8:(nn + 1) * 8, KEEP:P],
                            start=False, stop=True)
                    cp(G[wh][:, j, np_ * 16:(np_ + 1) * 16, :], ps[:], EV, "iW")

        # ---------------- turn4: Ght (128=k1s, (w 128, c 64)) --------------
        Ght = [sb.tile([P, P, BS], BF16, tag=f"tagCD{wh}", name=f"Ght{wh}_{b}")
               for wh in range(2)]
        for wh in range(2):
            for cq in range(16):
                pt = ptp.tile([P, 4, P], BF16, tag="tp", name=f"t4_{b}_{wh}_{cq}")
                for i in range(4):
                    # free slice (j 2, k1 64) -> out partitions [k1re | k1im]
                    nc.tensor.transpose(pt[:, i, :],
                                        G[wh][:, :, :, cq * 4 + i], ident[:])
                # scatter (c 4, w 128) psum into Ght free dims (w 128, c 4)
                dst = Ght[wh][:, :, cq * 4:(cq + 1) * 4]
                cp(dst, pt[:].rearrange("p c w -> p w c"), SK, "t4")

        # ---------------- invH + residual + store --------------------------
        # Tail balance: alternate drains between DVE tensor_add (residual on
        # the vector engine) and an identity-matmul residual accumulated in
        # PSUM (exact bf16 pass-through) drained by a plain ACT copy, so the
        # end-of-batch tail loads PE/DVE/ACT roughly evenly.
        for hc in range(2):
            for q8 in range(8):      # groups of 32 w
                ot = outp.tile([P, 32, BS], BF16, tag="ot",
                               name=f"ot_{b}_{hc}_{q8}")
                for k2 in range(2):  # 16-w halves of the tile
                    on_act = (q8 * 2 + k2) % 2 == 1
                    ps = pmm.tile([P, 16, BS], F32, tag="mm",
                                  name=f"psH_{b}_{hc}_{q8}_{k2}")
                    for k in range(2):
                        wg = q8 * 4 + k2 * 2 + k   # global 8-w group (0..31)
                        w8 = (wg % 16) * 8
                        nc.tensor.matmul(
                            ps[:, k * 8:(k + 1) * 8, :], LIH[hc],
                            Ght[wg // 16][:, w8:w8 + 8, :],
                            start=True, stop=not on_act)
                        if on_act:
                            rw = wg * 8
                            nc.tensor.matmul(
                                ps[:, k * 8:(k + 1) * 8, :], ident,
                                xt[hc][:, :, rw:rw + 8].rearrange(
                                    "p c w -> p w c"),
                                start=False, stop=True)
                    wbase = q8 * 32 + k2 * 16
                    dst = ot[:, k2 * 16:(k2 + 1) * 16, :]
                    if on_act:
                        nc.scalar.activation(out=dst, in_=ps[:], func=AF.Copy)
                    else:
                        res = xt[hc][:, :, wbase:wbase + 16].rearrange(
                            "p c w -> p w c")
                        nc.vector.tensor_add(out=dst, in0=ps[:], in1=res)
                nc.sync.dma_start(
                    out=out[b, hc * P:(hc + 1) * P,
                            q8 * 32:(q8 + 1) * 32, :],
                    in_=ot[:])


def _prepare_in_maps(x, w1, b1, w2, b2):
    bf = ml_dtypes.bfloat16
    ffwd, lbw, liw, lih = _host_consts()
    x = np.asarray(x, dtype=np.float32)

    in_maps = []
    for n in range(NB):
        xs = np.ascontiguousarray(
            x[..., n * BS:(n + 1) * BS].transpose(0, 1, 3, 2))
        w1n = np.asarray(w1[:, n], dtype=np.float32)   # (2,64,128)
        w2n = np.asarray(w2[:, n], dtype=np.float32)   # (2,128,64)
        b1n = np.asarray(b1[:, n], dtype=np.float32)   # (2,128)
        b2n = np.asarray(b2[:, n], dtype=np.float32)   # (2,64)
        m1 = np.stack([
            np.stack([w1n[0], -w1n[1]]),
            np.stack([w1n[1], w1n[0]]),
        ]).astype(bf)                                   # (2,2,64,128)
        m2 = np.stack([
            np.concatenate([w2n[0], w2n[1]], axis=1),
            np.concatenate([-w2n[1], w2n[0]], axis=1),
        ]).astype(bf)                                   # (2,128,128)
        in_maps.append({
            "xbf": xs.astype(bf),
            "ffwd": ffwd,
            "lbw": lbw,
            "m1": m1,
            "m2": m2,
            "b1s": b1n[:, :, None].copy(),
            "b2s": np.concatenate([b2n[0], b2n[1]])[:, None].copy(),
            "liw": liw,
            "lih": lih,
        })

    return in_maps


def kernel(x, w1, b1, w2, b2):
    global _CACHED_NC
    if _CACHED_NC is None:
        _CACHED_NC = _build_nc()
    nc = _CACHED_NC
    in_maps = _prepare_in_maps(x, w1, b1, w2, b2)
    res = run_bass_kernel_spmd(nc, in_maps, list(range(NB)))
    return np.concatenate(
        [res.results[i]["out"].astype(np.float32) for i in range(NB)], axis=-1)


# revision 25
# speedup vs baseline: 1.3176x; 1.3176x over previous
"""DPOT2D layer (AFNO-style) Trainium2 kernel.

out = x + irfft2_pad(blockMLP(trunc64(rfft2(x))))   (ortho norm)

Sharding: tensor-parallel over the 8 block-diagonal channel groups — core n
gets channels [n*64, (n+1)*64) and its block's MLP weights. Blocks never mix,
so there is zero cross-core communication.

Per core, every FFT stage is a DFT matmul on the TensorEngine (bf16 operands,
fp32 PSUM accumulation), with PE-transpose corner turns between stages:

  A:  V[w,(c,k1s)]   = x-slice^T @ F_h     (activation-stationary: x[h, c, w]
                                            slices are lhsT, contract h; no
                                            corner turn needed before B)
  B:  Y[k2s,(c,k1)]  = DFT_w on complex V  (re/im column accumulation)
  t2: Yt[c,(k1,k2s)] = corner turn of Y
  L1: o1 = gelu(W1 Y + b1)                 (K=64, re/im col accumulation)
  L2: O2[(o2r|o2i),(k1,k2)] = W2 o1 + b2   (K=128)
  t3: R[k2,(k1,o2s)] = corner turn of O2
  iW: G[w,(j,k1,c)]  = hermitian irfft_w matmuls (re/im col accumulation)
  t4: Ght[k1s,(w,c)] = corner turn of G (re/im interleaved -> k1-stack)
  iH: x'[h,(w,c)]    = E_h^T @ Ght, + residual x (bf16 from SBUF), DMA out

The residual is the bf16 x already resident in SBUF (loaded once per batch as
two whole-plane [128, 256, 64] tiles), so HBM traffic is 2x8.4MB in +
2x8.4MB out bf16 per core. Output DRAM is bf16; the host upcasts to f32.
PSUM->SBUF drains use 2-bank fp32 tiles (FD=1024) for matmul stages and
bf16 1-bank tiles (FD=512, 2x copy mode) for the corner turns, statically
balanced between DVE and ACT.
"""

import numpy as np
import ml_dtypes

import concourse.bass as bass
import concourse.mybir as mybir
from concourse import bacc
from concourse import masks
from concourse.tile import TileContext
from concourse.bass_utils import run_bass_kernel_spmd

B = 2
H = 256
W = 256
C = 512
NB = 8
BS = 64          # channels per block (= per core)
KEEP = 64        # kept modes per spatial dim
HID = 128
P = 128

BF16 = mybir.dt.bfloat16
F32 = mybir.dt.float32
AF = mybir.ActivationFunctionType

_CACHED_NC = None


def _host_consts():
    """DFT matrices shared by all cores (fp32 -> bf16)."""
    h = np.arange(H, dtype=np.float64)[:, None]
    k = np.arange(KEEP, dtype=np.float64)[None, :]
    th = 2.0 * np.pi * h * k / H
    F = np.concatenate([np.cos(th), -np.sin(th)], axis=1) / 16.0      # (256,128)
    Fwre, Fwim = F[:, :KEEP], F[:, KEEP:]
    lb_re = np.concatenate([Fwre, Fwim], axis=1)                      # (256,128)
    lb_im = np.concatenate([-Fwim, Fwre], axis=1)
    alpha = np.where(np.arange(KEEP) == 0, 1.0, 2.0)
    k2 = np.arange(KEEP, dtype=np.float64)[:, None]
    wv = np.arange(W, dtype=np.float64)[None, :]
    tw = 2.0 * np.pi * k2 * wv / W
    Ca = alpha[:, None] * np.cos(tw) / 16.0                           # (64,256)
    Sa = alpha[:, None] * np.sin(tw) / 16.0
    k1 = np.arange(KEEP, dtype=np.float64)[:, None]
    hv = np.arange(H, dtype=np.float64)[None, :]
    tih = 2.0 * np.pi * k1 * hv / H
    Ehc = np.cos(tih) / 16.0                                          # (64,256)
    Ehs = np.sin(tih) / 16.0
    lih_full = np.concatenate([Ehc, -Ehs], axis=0)                    # (128,256)

    bf = ml_dtypes.bfloat16
    ffwd = np.stack([F[0:128], F[128:256]]).astype(bf)                # (2,128,128)
    lbw = np.stack([
        np.stack([lb_re[0:128], lb_im[0:128]]),
        np.stack([lb_re[128:256], lb_im[128:256]]),
    ]).astype(bf)                                                     # (2,2,128,128)
    liw = np.stack([
        np.stack([np.stack([Ca[:, 0:128], -Sa[:, 0:128]]),
                  np.stack([Sa[:, 0:128], Ca[:, 0:128]])]),
        np.stack([np.stack([Ca[:, 128:256], -Sa[:, 128:256]]),
                  np.stack([Sa[:, 128:256], Ca[:, 128:256]])]),
    ]).astype(bf)                                                     # (2,2,2,64,128)
    lih = np.stack([lih_full[:, 0:128], lih_full[:, 128:256]]).astype(bf)  # (2,128,128)
    return ffwd, lbw, liw, lih


def _build_nc(loop_iters=0, probe=None):
    """loop_iters>0 wraps the whole per-batch pipeline in an on-device
    For_i repeat loop — used only by the timing harness to amortize the
    axon dispatch overhead out of the measurement.
    probe: None | 'dma' (DMAs only) | 'compute' (no input DMAs)."""
    nc = bacc.Bacc()

    xbf = nc.declare_dram_parameter("xbf", [B, H, BS, W], BF16, isOutput=False)
    ffwd_d = nc.declare_dram_parameter("ffwd", [2, P, P], BF16, isOutput=False)
    lbw_d = nc.declare_dram_parameter("lbw", [2, 2, P, P], BF16, isOutput=False)
    m1_d = nc.declare_dram_parameter("m1", [2, 2, BS, HID], BF16, isOutput=False)
    m2_d = nc.declare_dram_parameter("m2", [2, HID, P], BF16, isOutput=False)
    b1s_d = nc.declare_dram_parameter("b1s", [2, HID, 1], F32, isOutput=False)
    b2s_d = nc.declare_dram_parameter("b2s", [P, 1], F32, isOutput=False)
    liw_d = nc.declare_dram_parameter("liw", [2, 2, 2, KEEP, P], BF16, isOutput=False)
    lih_d = nc.declare_dram_parameter("lih", [2, P, P], BF16, isOutput=False)
    out = nc.declare_dram_parameter("out", [B, H, W, BS], BF16, isOutput=True)

    with TileContext(nc) as tc:
        consts = tc.alloc_tile_pool(name="consts", bufs=1)
        ident = consts.tile([P, P], BF16, name="ident")
        masks.make_identity(nc, ident[:])

        def const2d(name, dram_ap, shape, dtype=BF16):
            t = consts.tile(shape, dtype, name=name)
            nc.sync.dma_start(out=t[:], in_=dram_ap)
            return t

        FW = [const2d(f"fw{hh}", ffwd_d[hh], [P, P]) for hh in range(2)]
        LBW = [[const2d(f"lbw{wh}{s}", lbw_d[wh, s], [P, P]) for s in range(2)]
               for wh in range(2)]
        M1 = [[const2d(f"m1_{j}{s}", m1_d[j, s], [BS, HID]) for s in range(2)]
              for j in range(2)]
        M2 = [const2d(f"m2_{s}", m2_d[s], [HID, P]) for s in range(2)]
        LIW = [[[const2d(f"liw{wh}{j}{s}", liw_d[wh, j, s], [KEEP, P])
                 for s in range(2)] for j in range(2)] for wh in range(2)]
        LIH = [const2d(f"lih{hc}", lih_d[hc], [P, P]) for hc in range(2)]
        b1s_t = [const2d(f"b1s{j}", b1s_d[j], [HID, 1], F32) for j in range(2)]
        b2s_t = const2d("b2s", b2s_d[:], [P, 1], F32)

        # PSUM->SBUF drain with static DVE/ACT balance. kind:
        #   'copy'  — plain copy (tensor_copy / activation Copy)
        #   engines chosen by per-stage pattern strings of 'D'/'A'.
        def mk_cp():
            state = {}

            def cp(dst, src, pat, key):
                i = state.get(key, 0)
                state[key] = i + 1
                if pat[i % len(pat)] == "D":
                    nc.vector.tensor_copy(out=dst, in_=src)
                else:
                    nc.scalar.activation(out=dst, in_=src, func=AF.Copy)
            return cp

        cp = mk_cp()

        sb = tc.alloc_tile_pool(name="sb", bufs=1)
        xtp = tc.alloc_tile_pool(name="xtp", bufs=1)
        outp = tc.alloc_tile_pool(name="outp", bufs=4)
        pmm = tc.alloc_tile_pool(name="pmm", bufs=2, space="PSUM")
        ptp = tc.alloc_tile_pool(name="ptp", bufs=4, space="PSUM")

        import contextlib
        loop_ctx = tc.For_i(0, loop_iters, 1) if loop_iters else contextlib.nullcontext()
        with loop_ctx:
            if probe == "dma":
                _emit_dma_probe(nc, tc, locals())
            else:
                _emit_body(nc, tc, locals(), skip_dma=(probe == "compute"))
        ptp.release()
        pmm.release()
        outp.release()
        xtp.release()
        sb.release()
        consts.release()
    nc.compile()
    return nc


def _emit_dma_probe(nc, tc, env):
    """Same DMA traffic as the real kernel (x-in bf16 whole planes, out bf16),
    no compute: out tiles are fed from the x tiles."""
    xbf = env["xbf"]; out = env["out"]
    xtp = env["xtp"]; outp = env["outp"]
    for b in range(B):
        xt = []
        for hh in range(2):
            t = xtp.tile([P, BS, W], BF16, tag=f"xt{hh}", name=f"pxt{hh}_{b}")
            nc.sync.dma_start(out=t[:], in_=xbf[b, hh * P:(hh + 1) * P, :, :])
            xt.append(t)
        for hc in range(2):
            for q8 in range(8):
                ot = outp.tile([P, 32, BS], BF16, tag="ot",
                               name=f"pot_{b}_{hc}_{q8}")
                nc.vector.tensor_copy(
                    out=ot[:],
                    in_=xt[hc][:, :, q8 * 32:(q8 + 1) * 32].rearrange(
                        "p c w -> p w c"))
                nc.sync.dma_start(
                    out=out[b, hc * P:(hc + 1) * P, q8 * 32:(q8 + 1) * 32, :],
                    in_=ot[:])


def _emit_body(nc, tc, env, skip_dma=False):
    xbf = env["xbf"]; out = env["out"]
    FW = env["FW"]; LBW = env["LBW"]; M1 = env["M1"]; M2 = env["M2"]
    LIW = env["LIW"]; LIH = env["LIH"]; b1s_t = env["b1s_t"]; b2s_t = env["b2s_t"]
    ident = env["ident"]; cp = env["cp"]
    sb = env["sb"]; xtp = env["xtp"]; outp = env["outp"]
    pmm = env["pmm"]; ptp = env["ptp"]

    EV = "A"             # matmul-stage drains: ACT (DVE is loaded by iH adds)
    SK = "DADDA"         # turn drains: 3 DVE / 2 ACT (bf16 src -> DVE 2x mode)

    for b in range(B):
        # ---------------- load x: two whole (c,w) planes ----------------
        xt = []
        for hh in range(2):
            t = xtp.tile([P, BS, W], BF16, tag=f"xt{hh}", name=f"xt{hh}_{b}")
            if not skip_dma:
                nc.sync.dma_start(out=t[:], in_=xbf[b, hh * P:(hh + 1) * P, :, :])
            else:
                nc.sync.dma_start(out=t[0:1, 0:1, :], in_=xbf[b, 0:1, 0:1, :])
            xt.append(t)

        # ------- stage A (act-stationary): V[wh] (128=w, (c 64, k1s 128)) --
        # lhsT = x[h, c, w-half] slice (contract h), rhs = F_h chunk.
        V = [sb.tile([P, BS, P], BF16, tag=f"tagCD{wh}", name=f"V{wh}_{b}")
             for wh in range(2)]
        for wh in range(2):
            for cq in range(8):      # 8 c per psum tile
                ps = pmm.tile([P, 8, P], F32, tag="mm", name=f"psA_{b}_{wh}_{cq}")
                for i in range(8):
                    c = cq * 8 + i
                    for hh in range(2):
                        nc.tensor.matmul(
                            ps[:, i, :],
                            xt[hh][:, c, wh * P:(wh + 1) * P], FW[hh],
                            start=(hh == 0), stop=(hh == 1))
                cp(V[wh][:, cq * 8:(cq + 1) * 8, :], ps[:], EV, "A")

        # ---------------- stage B: Y (128=k2s, (c 64, k1 64)) --------------
        Y = sb.tile([P, BS, KEEP], BF16, tag="tagE", name=f"Y_{b}")
        for np_ in range(4):         # 16 c per psum tile
            ps = pmm.tile([P, 16, KEEP], F32, tag="mm", name=f"psB_{b}_{np_}")
            for k in range(2):
                nn = np_ * 2 + k     # 8-c chunk
                first = True
                for wh in range(2):
                    for s in range(2):
                        rhs = V[wh][:, nn * 8:(nn + 1) * 8,
                                    s * KEEP:(s + 1) * KEEP]
                        nc.tensor.matmul(ps[:, k * 8:(k + 1) * 8, :],
                                         LBW[wh][s], rhs,
                                         start=first, stop=(wh == 1 and s == 1))
                        first = False
            cp(Y[:, np_ * 16:(np_ + 1) * 16, :], ps[:], EV, "B")

        # ---------------- turn2: Yt (64=c, (k1 64, k2s 128)) ---------------
        Yt = sb.tile([BS, KEEP, P], BF16, tag="tagF", name=f"Yt_{b}")
        for kq in range(16):         # groups of 4 k1
            pt = ptp.tile([P, 4, P], BF16, tag="tp", name=f"t2_{b}_{kq}")
            for i in range(4):
                nc.tensor.transpose(pt[0:BS, i, :], Y[:, :, kq * 4 + i],
                                    ident[:])
            cp(Yt[:, kq * 4:(kq + 1) * 4, :], pt[0:BS, :, :], SK, "t2")

        # ---------------- MLP L1 (K=64) + gelu -----------------------------
        o1 = [sb.tile([HID, KEEP, KEEP], BF16, tag=f"o1_{j}", name=f"o1_{j}_{b}")
              for j in range(2)]
        for j in range(2):
            for np_ in range(4):     # 16 k1 per psum tile
                ps = pmm.tile([HID, 16, KEEP], F32, tag="mm",
                              name=f"ps1_{b}_{j}_{np_}")
                for k in range(2):
                    nn = np_ * 2 + k
                    nc.tensor.matmul(
                        ps[:, k * 8:(k + 1) * 8, :], M1[j][0],
                        Yt[:, nn * 8:(nn + 1) * 8, 0:KEEP],
                        start=True, stop=False)
                    nc.tensor.matmul(
                        ps[:, k * 8:(k + 1) * 8, :], M1[j][1],
                        Yt[:, nn * 8:(nn + 1) * 8, KEEP:P],
                        start=False, stop=True)
                nc.scalar.activation(
                    out=o1[j][:, np_ * 16:(np_ + 1) * 16, :],
                    in_=ps[:], func=AF.Gelu, bias=b1s_t[j][:])

        # ---------------- MLP L2 (K=128) + bias ----------------------------
        O2 = sb.tile([P, KEEP, KEEP], BF16, tag="tagF2", name=f"O2_{b}")
        for np_ in range(4):
            ps = pmm.tile([P, 16, KEEP], F32, tag="mm", name=f"ps2_{b}_{np_}")
            for k in range(2):
                nn = np_ * 2 + k
                nc.tensor.matmul(ps[:, k * 8:(k + 1) * 8, :], M2[0],
                                 o1[0][:, nn * 8:(nn + 1) * 8, :],
                                 start=True, stop=False)
                nc.tensor.matmul(ps[:, k * 8:(k + 1) * 8, :], M2[1],
                                 o1[1][:, nn * 8:(nn + 1) * 8, :],
                                 start=False, stop=True)
            nc.scalar.activation(
                out=O2[:, np_ * 16:(np_ + 1) * 16, :], in_=ps[:],
                func=AF.Identity, bias=b2s_t[:])

        # ---------------- turn3: R (64=k2, (k1 64, o2s 128)) ---------------
        R = sb.tile([KEEP, KEEP, P], BF16, tag="tagE", name=f"R_{b}")
        for kq in range(16):
            pt = ptp.tile([P, 4, P], BF16, tag="tp", name=f"t3_{b}_{kq}")
            for i in range(4):
                nc.tensor.transpose(pt[0:KEEP, i, :], O2[:, kq * 4 + i, :],
                                    ident[:])
            cp(R[:, kq * 4:(kq + 1) * 4, :], pt[0:KEEP, :, :], SK, "t3")

        # ---------------- invW: G[wh] (128=w, (j 2, k1 64, c 64)) ----------
        G = [sb.tile([P, 2, KEEP, BS], BF16, tag=f"tagAB{wh}", name=f"G{wh}_{b}")
             for wh in range(2)]
        for wh in range(2):
            for j in range(2):       # 0: Gre, 1: Gim
                for np_ in range(4):
                    ps = pmm.tile([P, 16, BS], F32, tag="mm",
                                  name=f"psW_{b}_{wh}_{j}_{np_}")
                    for k in range(2):
                        nn = np_ * 2 + k
                        nc.tensor.matmul(
                            ps[:, k * 8:(k + 1) * 8, :], LIW[wh][j][0],
                            R[:, nn * 8:(nn + 1) * 8, 0:KEEP],
                            start=True, stop=False)
                        nc.tensor.matmul(
                            ps[:, k * 8:(k + 1) * 8, :], LIW[wh][j][1],
                            R[:, nn * 8:(nn + 1) * 8, KEEP:P],
                            start=False, stop=True)
                    cp(G[wh][:, j, np_ * 16:(np_ + 1) * 16, :], ps[:], EV, "iW")

        # ---------------- turn4: Ght (128=k1s, (w 128, c 64)) --------------
        Ght = [sb.tile([P, P, BS], BF16, tag=f"tagCD{wh}", name=f"Ght{wh}_{b}")
               for wh in range(2)]
        for wh in range(2):
            for cq in range(16):
                pt = ptp.tile([P, 4, P], BF16, tag="tp", name=f"t4_{b}_{wh}_{cq}")
                for i in range(4):
                    # free slice (j 2, k1 64) -> out partitions [k1re | k1im]
                    nc.tensor.transpose(pt[:, i, :],
                                        G[wh][:, :, :, cq * 4 + i], ident[:])
                # scatter (c 4, w 128) psum into Ght free dims (w 128, c 4)
                dst = Ght[wh][:, :, cq * 4:(cq + 1) * 4]
                cp(dst, pt[:].rearrange("p c w -> p w c"), SK, "t4")

        # ---------------- invH + residual + store --------------------------
        # Tail balance: alternate drains between DVE tensor_add (residual on
        # the vector engine) and an identity-matmul residual accumulated in
        # PSUM (exact bf16 pass-through) drained by a plain ACT copy, so the
        # end-of-batch tail loads PE/DVE/ACT roughly evenly.
        for hc in range(2):
            for q8 in range(8):      # groups of 32 w
                ot = outp.tile([P, 32, BS], BF16, tag="ot",
                               name=f"ot_{b}_{hc}_{q8}")
                for k2 in range(2):  # 16-w halves of the tile
                    on_act = (q8 * 2 + k2) % 2 == 1
                    ps = pmm.tile([P, 16, BS], F32, tag="mm",
                                  name=f"psH_{b}_{hc}_{q8}_{k2}")
                    for k in range(2):
                        wg = q8 * 4 + k2 * 2 + k   # global 8-w group (0..31)
                        w8 = (wg % 16) * 8
                        nc.tensor.matmul(
                            ps[:, k * 8:(k + 1) * 8, :], LIH[hc],
                            Ght[wg // 16][:, w8:w8 + 8, :],
                            start=True, stop=not on_act)
                        if on_act:
                            rw = wg * 8
                            nc.tensor.matmul(
                                ps[:, k * 8:(k + 1) * 8, :], ident,
                                xt[hc][:, :, rw:rw + 8].rearrange(
                                    "p c w -> p w c"),
                                start=False, stop=True)
                    wbase = q8 * 32 + k2 * 16
                    dst = ot[:, k2 * 16:(k2 + 1) * 16, :]
                    if on_act:
                        nc.scalar.activation(out=dst, in_=ps[:], func=AF.Copy)
                    else:
                        res = xt[hc][:, :, wbase:wbase + 16].rearrange(
                            "p c w -> p w c")
                        nc.vector.tensor_add(out=dst, in0=ps[:], in1=res)
                nc.sync.dma_start(
                    out=out[b, hc * P:(hc + 1) * P,
                            q8 * 32:(q8 + 1) * 32, :],
                    in_=ot[:])


def _prepare_in_maps(x, w1, b1, w2, b2):
    bf = ml_dtypes.bfloat16
    ffwd, lbw, liw, lih = _host_consts()
    x = np.asarray(x, dtype=np.float32)

    in_maps = []
    for n in range(NB):
        xs = np.ascontiguousarray(
            x[..., n * BS:(n + 1) * BS].transpose(0, 1, 3, 2))
        w1n = np.asarray(w1[:, n], dtype=np.float32)   # (2,64,128)
        w2n = np.asarray(w2[:, n], dtype=np.float32)   # (2,128,64)
        b1n = np.asarray(b1[:, n], dtype=np.float32)   # (2,128)
        b2n = np.asarray(b2[:, n], dtype=np.float32)   # (2,64)
        m1 = np.stack([
            np.stack([w1n[0], -w1n[1]]),
            np.stack([w1n[1], w1n[0]]),
        ]).astype(bf)                                   # (2,2,64,128)
        m2 = np.stack([
            np.concatenate([w2n[0], w2n[1]], axis=1),
            np.concatenate([-w2n[1], w2n[0]], axis=1),
        ]).astype(bf)                                   # (2,128,128)
        in_maps.append({
            "xbf": xs.astype(bf),
            "ffwd": ffwd,
            "lbw": lbw,
            "m1": m1,
            "m2": m2,
            "b1s": b1n[:, :, None].copy(),
            "b2s": np.concatenate([b2n[0], b2n[1]])[:, None].copy(),
            "liw": liw,
            "lih": lih,
        })

    return in_maps


def kernel(x, w1, b1, w2, b2):
    global _CACHED_NC
    if _CACHED_NC is None:
        _CACHED_NC = _build_nc()
    nc = _CACHED_NC
    in_maps = _prepare_in_maps(x, w1, b1, w2, b2)
    res = run_bass_kernel_spmd(nc, in_maps, list(range(NB)))
    return np.concatenate(
        [res.results[i]["out"].astype(np.float32) for i in range(NB)], axis=-1)
